# revision 18
# baseline (speedup 1.0000x reference)
"""Last-query sparse attention on 8 TRN2 NeuronCores.

Reference computation (per sample b):
    prev  = x[b, :-1, :]                 # [T-1, D]
    final = x[b, -1, :]                  # [D]
    s     = prev @ final                 # [T-1]
    w     = softmax(s)
    att   = w @ prev                     # [D]
    out   = concat(final, att)           # [2D]

Sharding: batch (B=64) split 8 ways -> 8 samples per core, no collectives.

Per-core layout: x[b] ([4096, 256] f32) is DMA'd to SBUF as [128, 32, 256]
with partition p holding rows t = p*32 + i (32KB contiguous HBM per
partition -> efficient descriptors).

Pass 1 (scores, contraction over free dim d): DVE tensor_tensor_reduce per
block i: accum[p] = sum_d x[p,i,d] * final[d] -> S[128, 32].
The self-score at t=4095 (p=127, i=31) is masked to -1e30.

Softmax: DVE row max -> GPSIMD partition_all_reduce(max) -> ACT exp with
per-partition bias (-gmax), fused row-sum accumulation; denominator via a
tiny PE matmul against a ones vector.

Pass 2 (weighted sum, contraction over t on partitions): PE matmuls
accumulating in PSUM: lhsT = exp-weights column [128, 1], rhs = x block
[128, 256] viewed as float32r (full-rate fp32 streaming), over 32 blocks.
Normalize by 1/Z on DVE, then DMA concat(final, att) to the output.
"""

import sys

sys.path.insert(0, "/opt/trn_rl_repo")

from contextlib import ExitStack

import numpy as np

import concourse.tile as tile
from concourse import bacc, mybir
from concourse.bass_utils import run_bass_kernel_spmd
from concourse.masks import make_identity

N_CORES = 8
B = 64
T = 4096
D = 256
BPC = B // N_CORES  # samples per core
P = 128
NBLK = T // P  # 32 blocks; t = p*NBLK + i
F32 = mybir.dt.float32
BF16 = mybir.dt.bfloat16

_NC_CACHE = None


def _build():
    nc = bacc.Bacc(
        trn_type="TRN2",
        target_bir_lowering=False,
        debug=False,
        num_devices=N_CORES,
    )
    x_ext = nc.declare_dram_parameter("x", [BPC, T, D], F32, isOutput=False)
    out_ext = nc.declare_dram_parameter("out", [BPC, 2 * D], F32, isOutput=True)
    xap = x_ext.ap()
    oap = out_ext.ap()

    with ExitStack() as ctx:
        tc = ctx.enter_context(tile.TileContext(nc))
        xpool = ctx.enter_context(tc.tile_pool(name="xp", bufs=3))
        xbpool = ctx.enter_context(tc.tile_pool(name="xbp", bufs=2))
        fpool = ctx.enter_context(tc.tile_pool(name="fp", bufs=3))
        scrpool = ctx.enter_context(tc.tile_pool(name="scr", bufs=2))
        spool = ctx.enter_context(tc.tile_pool(name="sp", bufs=2))
        stat = ctx.enter_context(tc.tile_pool(name="stat", bufs=4))
        cpool = ctx.enter_context(tc.tile_pool(name="const", bufs=1))
        opool = ctx.enter_context(tc.tile_pool(name="outp", bufs=2))
        pspool = ctx.enter_context(tc.tile_pool(name="ps", bufs=2, space="PSUM"))
        statps = ctx.enter_context(tc.tile_pool(name="sps", bufs=6, space="PSUM"))

        ones = cpool.tile([P, 1], F32)
        nc.gpsimd.memset(ones[:], 1.0)
        neg_ones_row = cpool.tile([1, P], F32)
        nc.gpsimd.memset(neg_ones_row[:], -1.0)
        identity = cpool.tile([P, P], F32)
        make_identity(nc, identity[:])

        # maskbias[p] = -1e30 if p == 127 else 0 (masks the query's
        # self-score without touching a partition-127-based AP)
        pidx = cpool.tile([P, 1], mybir.dt.int32)
        nc.gpsimd.iota(pidx[:], pattern=[[0, 1]], base=0, channel_multiplier=1)
        maskbias = cpool.tile([P, 1], F32)
        nc.vector.tensor_scalar(
            out=maskbias[:],
            in0=pidx[:],
            scalar1=126,
            scalar2=None,
            op0=mybir.AluOpType.is_gt,
        )
        nc.vector.tensor_scalar_mul(maskbias[:], maskbias[:], -1.0e30)

        for b in range(BPC):
            X = xpool.tile([P, NBLK, D], F32)
            nc.sync.dma_start(X[:], xap[b].rearrange("(p i) d -> p i d", p=P))
            F = fpool.tile([P, D], F32)
            nc.sync.dma_start(F[:], xap[b, T - 1].partition_broadcast(P))

            # bf16 copy of x for the pass-2 matmuls (PE streams bf16 at full
            # rate; fp32 would cost 4 cycles/row). ACT is otherwise idle.
            Xb = xbpool.tile([P, NBLK, D], BF16)
            nc.scalar.copy(Xb[:], X[:])

            S = spool.tile([P, NBLK], F32)
            scr = scrpool.tile([P, D], F32)
            for i in range(NBLK):
                # fused multiply + free-dim reduce on DVE:
                # scr = (X*1) * F ; S[:, i] = sum(scr)
                nc.vector.scalar_tensor_tensor(
                    out=scr[:],
                    in0=X[:, i, :],
                    scalar=1.0,
                    in1=F[:],
                    op0=mybir.AluOpType.mult,
                    op1=mybir.AluOpType.mult,
                    accum_out=S[:, i : i + 1],
                )
            # mask the query's self-score (t = 4095 -> p=127, i=31)
            nc.vector.tensor_add(
                S[:, NBLK - 1 : NBLK], S[:, NBLK - 1 : NBLK], maskbias[:]
            )

            rowmax = stat.tile([P, 1], F32)
            nc.vector.reduce_max(rowmax[:], S[:], axis=mybir.AxisListType.X)
            # cross-partition max: PE transpose -> free-dim max -> PE
            # broadcast back to all partitions with a -1 weight (fuses the
            # negation needed for the exp bias)
            rmT = statps.tile([1, P], F32, tag="sps")
            nc.tensor.transpose(rmT[:], rowmax[:], identity[:])
            gmax = stat.tile([1, 1], F32)
            nc.vector.reduce_max(gmax[:], rmT[:], axis=mybir.AxisListType.X)
            negb = statps.tile([P, 1], F32, tag="sps")
            nc.tensor.matmul(
                negb[:], lhsT=neg_ones_row[:], rhs=gmax[:], start=True, stop=True
            )
            negmax = stat.tile([P, 1], F32)
            nc.vector.tensor_copy(negmax[:], negb[:])

            Pw = spool.tile([P, NBLK], BF16)
            rowsum = stat.tile([P, 1], F32)
            nc.scalar.activation(
                Pw[:],
                S[:],
                mybir.ActivationFunctionType.Exp,
                bias=negmax[:],
                scale=1.0,
                accum_out=rowsum[:],
            )

            Zp = statps.tile([1, 1], F32, tag="sps")
            nc.tensor.matmul(Zp[:], lhsT=rowsum[:], rhs=ones[:], start=True, stop=True)

            att = pspool.tile([1, D], F32)
            for i in range(NBLK):
                nc.tensor.matmul(
                    att[:],
                    lhsT=Pw[:, i : i + 1],
                    rhs=Xb[:, i, :],
                    start=(i == 0),
                    stop=(i == NBLK - 1),
                )

            rz = stat.tile([1, 1], F32)
            nc.vector.reciprocal(rz[:], Zp[:])
            att_sb = opool.tile([1, D], F32)
            nc.vector.tensor_scalar_mul(att_sb[:], att[:], rz[:])

            nc.sync.dma_start(oap[b : b + 1, 0:D], F[0:1, :])
            nc.sync.dma_start(oap[b : b + 1, D : 2 * D], att_sb[:])

    nc.compile()
    return nc


def _run(x, trace=False):
    global _NC_CACHE
    x = np.ascontiguousarray(np.asarray(x, dtype=np.float32))
    assert x.shape == (B, T, D), x.shape
    if _NC_CACHE is None:
        _NC_CACHE = _build()
    in_maps = [{"x": x[c * BPC : (c + 1) * BPC]} for c in range(N_CORES)]
    res = run_bass_kernel_spmd(
        _NC_CACHE, in_maps, core_ids=list(range(N_CORES)), trace=trace
    )
    out = np.concatenate([res.results[c]["out"] for c in range(N_CORES)], axis=0)
    return out.astype(np.float32), res


def kernel(x):
    out, _ = _run(x, trace=False)
    return out
